# revision 1
# baseline (speedup 1.0000x reference)
"""GCN joint-representation edge MLP on 8 TRN2 NeuronCores (Bass/Tile).

reference:
    node_rep = z[edge_index[0]] * z[edge_index[1]]          # [E, 64]
    joint    = concat([node_rep, edge_attr], -1)            # [E, 832]
    h        = relu(joint @ W1 + b1)                        # [E, 128]
    out      = softmax(h @ W2 + b2, -1)                     # [E, 5]

Sharding: pure data-parallel over edges, 8 cores x 25088 edges (E padded
200000 -> 200704).  Each core streams its edge slice and runs the full
MLP + softmax on device.

Layout choices made during host-side sharding:
  - endpoint z-rows are resolved to dense per-edge streams (z[src], z[dst]).
    Device-side row-gather primitives are unusable in this runtime
    (multi-offset indirect DMA returns wrong data on HW; the dma_gather
    GPSIMD ucode crashes the exec unit; per-128-row indirect DMA costs
    1.6us/call = 3x the whole kernel budget).  The dense streams carry
    byte-for-byte the same device traffic as an on-device gather would.
  - all per-edge streams are stored feature-major (transposed): matmul
    operands DMA straight into [K, N] layout, the output is written
    class-major [5, E] — zero on-chip transposes, contiguous DMA runs.
  - zsT/zdT are stacked into one [128, E] stream (full-port DMA).
  - TensorE runs fp32r for the edge_attr chunks (full rate at N=512,
    ~1e-4 relerr); the node_rep chunk and layer 2 run bf16.

Device pipeline per 512-edge block (all edges in natural order):
  - DMA attrT [128, 6, 512] f32r (SP ring), zzT [128, 512] f32 (ACT ring)
  - node_rep = zzT[0:64]*zzT[64:128] (DVE, bf16 out)     [64, 512]
  - 7 accumulating matmuls -> hT (PSUM f32)              [128, 512]
  - ScalarE relu(+b1) -> hT bf16
  - matmul lhsT=W2 rhs=hT -> logitsT (PSUM f32)          [5, 512]
  - ScalarE exp(logitsT + b2) -> bf16                    [5, 512]
  - matmul lhsT=ones[5,1] -> class sums (PSUM f32)       [1, 512]
  - DVE reciprocal + partition-broadcast multiply -> probsT [5, 512] f32
  - DMA probsT -> outT[5, E] (ACT ring)
"""
import numpy as np

import concourse.bass as bass
import concourse.bacc as bacc
import concourse.tile as tile
from concourse import mybir
from concourse.bass_utils import run_bass_kernel_spmd

F32 = mybir.dt.float32
F32R = mybir.dt.float32r
BF16 = mybir.dt.bfloat16

N_CORES = 8
E_FULL = 200000
E_PAD = 200704              # 8 * 25088
E_CORE = E_PAD // N_CORES   # 25088 = 49 * 512
BLK = 512
NBLK = E_CORE // BLK        # 49
ZD = 64
AD = 768
NSL = AD // 128             # 6 attr feature slices
HID = 128
NCLS = 5


def build_nc(nblk=NBLK, reps=1):
    """Per-core Bass program (same NEFF on all 8 cores).  `reps` wraps the
    block loop with a For_i for timing runs."""
    nc = bacc.Bacc("TRN2", target_bir_lowering=False, debug=False)

    ecore = nblk * BLK
    attrT = nc.declare_dram_parameter("attrT", [AD, ecore], F32, isOutput=False)
    zzT = nc.declare_dram_parameter("zzT", [ZD, 2 * ecore], F32, isOutput=False)
    w1a = nc.declare_dram_parameter("w1a", [ZD, HID], BF16, isOutput=False)
    w1f = nc.declare_dram_parameter("w1f", [128, NSL, HID], F32, isOutput=False)
    w2 = nc.declare_dram_parameter("w2", [HID, NCLS], BF16, isOutput=False)
    b1 = nc.declare_dram_parameter("b1", [HID, 1], F32, isOutput=False)
    b2c = nc.declare_dram_parameter("b2c", [NCLS, 1], F32, isOutput=False)
    outT = nc.declare_dram_parameter("outT", [NCLS, ecore], F32, isOutput=True)

    attrT_v = attrT[:, :].rearrange("(s p) (b e) -> b p s e", p=128, e=BLK)
    zzT_v = zzT[:, :].rearrange("p (b e) -> b p e", e=2 * BLK)
    outT_v = outT[:, :].rearrange("p (b e) -> b p e", e=BLK)

    with tile.TileContext(nc) as tc:
        with (
            tc.tile_pool(name="const", bufs=1) as constp,
            tc.tile_pool(name="attrp", bufs=3) as attrp,
            tc.tile_pool(name="zp", bufs=3) as zp,
            tc.tile_pool(name="nrp", bufs=2) as nrp,
            tc.tile_pool(name="htp", bufs=2) as htp,
            tc.tile_pool(name="exp_", bufs=3) as expp,
            tc.tile_pool(name="outp", bufs=3) as outp,
            tc.tile_pool(name="ps_ht", bufs=2, space="PSUM") as ps_ht,
            tc.tile_pool(name="ps_lg", bufs=2, space="PSUM") as ps_lg,
            tc.tile_pool(name="ps_sum", bufs=2, space="PSUM") as ps_sum,
        ):
            # ---- constants ----
            w1a_t = constp.tile([ZD, HID], BF16)
            nc.sync.dma_start(out=w1a_t[:], in_=w1a[:, :])
            w1f_t = constp.tile([128, NSL, HID], F32R)
            nc.sync.dma_start(out=w1f_t[:], in_=w1f[:, :, :].bitcast(F32R))
            w2_t = constp.tile([HID, NCLS], BF16)
            nc.sync.dma_start(out=w2_t[:], in_=w2[:, :])
            b1_t = constp.tile([HID, 1], F32)
            nc.sync.dma_start(out=b1_t[:], in_=b1[:, :])
            b2_t = constp.tile([NCLS, 1], F32)
            nc.sync.dma_start(out=b2_t[:], in_=b2c[:, :])
            ones_t = constp.tile([NCLS, 1], BF16)
            nc.vector.memset(ones_t[:], 1.0)
            ones1_t = constp.tile([1, NCLS], F32)
            nc.vector.memset(ones1_t[:], 1.0)

            def body(b):
                attr_t = attrp.tile([128, NSL, BLK], F32R, tag="attr")
                nc.sync.dma_start(out=attr_t[:], in_=attrT_v[b].bitcast(F32R))
                zz_t = zp.tile([ZD, 2 * BLK], F32, tag="zz")
                nc.scalar.dma_start(out=zz_t[:], in_=zzT_v[b])

                nr_t = nrp.tile([ZD, BLK], BF16, tag="nr")
                nc.vector.tensor_mul(nr_t[:], zz_t[:, 0:BLK], zz_t[:, BLK:2 * BLK])

                # ---- layer 1: hT[128, 512] ----
                ht_ps = ps_ht.tile([HID, BLK], F32, tag="htps")
                nc.tensor.matmul(
                    out=ht_ps[:], lhsT=w1a_t[:], rhs=nr_t[:],
                    start=True, stop=False,
                )
                for s in range(NSL):
                    nc.tensor.matmul(
                        out=ht_ps[:], lhsT=w1f_t[:, s, :], rhs=attr_t[:, s, :],
                        start=False, stop=(s == NSL - 1),
                    )

                # ---- relu(+b1) -> hT bf16 ----
                ht_s = htp.tile([HID, BLK], BF16, tag="hts")
                nc.scalar.activation(
                    out=ht_s[:], in_=ht_ps[:],
                    func=mybir.ActivationFunctionType.Relu,
                    bias=b1_t[:],
                )

                # ---- layer 2: logitsT [5, 512] ----
                lg_ps = ps_lg.tile([NCLS, BLK], F32, tag="lgps")
                nc.tensor.matmul(
                    out=lg_ps[:], lhsT=w2_t[:], rhs=ht_s[:],
                    start=True, stop=True,
                )
                # exp(logits + b2) -> bf16
                ex_t = expp.tile([NCLS, BLK], BF16, tag="ex")
                nc.scalar.activation(
                    out=ex_t[:], in_=lg_ps[:],
                    func=mybir.ActivationFunctionType.Exp,
                    bias=b2_t[:],
                )
                # class sums via ones-matmul -> [1, 512]
                sum_ps = ps_sum.tile([1, BLK], F32, tag="sumps")
                nc.tensor.matmul(
                    out=sum_ps[:], lhsT=ones_t[:], rhs=ex_t[:],
                    start=True, stop=True,
                )
                rec = expp.tile([1, BLK], F32, tag="rec")
                nc.vector.reciprocal(out=rec[:], in_=sum_ps[:])
                # broadcast rec across the 5 class partitions via K=1 matmul
                rec5_ps = ps_sum.tile([NCLS, BLK], F32, tag="rec5")
                nc.tensor.matmul(
                    out=rec5_ps[:], lhsT=ones1_t[:], rhs=rec[:],
                    start=True, stop=True,
                )
                pr_t = outp.tile([NCLS, BLK], F32, tag="pr")
                nc.vector.tensor_mul(pr_t[:], ex_t[:], rec5_ps[:])
                nc.scalar.dma_start(out=outT_v[b], in_=pr_t[:])

            if reps == 1:
                for b in range(nblk):
                    body(b)
            else:
                with tc.For_i(0, reps, 1):
                    for b in range(nblk):
                        body(b)

    nc.compile()
    return nc


def _shard_inputs(z, edge_index, edge_attr, W1, b1, W2, b2):
    import ml_dtypes
    z = np.asarray(z, dtype=np.float32)
    ei = np.asarray(edge_index).astype(np.int64)
    attr = np.asarray(edge_attr, dtype=np.float32)
    W1 = np.asarray(W1, dtype=np.float32)
    b1 = np.asarray(b1, dtype=np.float32)
    W2 = np.asarray(W2, dtype=np.float32)
    b2 = np.asarray(b2, dtype=np.float32)

    src = np.zeros(E_PAD, dtype=np.int64)
    dst = np.zeros(E_PAD, dtype=np.int64)
    src[:E_FULL] = ei[0]
    dst[:E_FULL] = ei[1]

    # dense per-edge endpoint streams, feature-major, per-block [zs512|zd512]
    nblk_tot = E_PAD // BLK
    zzT = np.empty((ZD, nblk_tot, 2, BLK), dtype=np.float32)
    zzT[:, :, 0, :] = z[src].T.reshape(ZD, nblk_tot, BLK)
    zzT[:, :, 1, :] = z[dst].T.reshape(ZD, nblk_tot, BLK)
    zzT = zzT.reshape(ZD, 2 * E_PAD)
    attrT = np.zeros((AD, E_PAD), dtype=np.float32)
    attrT[:, :E_FULL] = attr.T

    w1a = W1[:ZD].astype(ml_dtypes.bfloat16)   # [64, 128] node_rep rows
    w1f = np.ascontiguousarray(
        W1[ZD:].reshape(NSL, 128, HID).transpose(1, 0, 2))  # [128, 6, 128]
    w2b = W2.astype(ml_dtypes.bfloat16)
    b1c = b1.reshape(HID, 1)
    b2c = b2.reshape(NCLS, 1)

    in_maps = []
    for c in range(N_CORES):
        s = slice(c * E_CORE, (c + 1) * E_CORE)
        s2 = slice(2 * c * E_CORE, 2 * (c + 1) * E_CORE)
        in_maps.append({
            "attrT": np.ascontiguousarray(attrT[:, s]),
            "zzT": np.ascontiguousarray(zzT[:, s2]),
            "w1a": w1a,
            "w1f": w1f,
            "w2": w2b,
            "b1": b1c,
            "b2c": b2c,
        })
    return in_maps


def kernel(z, edge_index, edge_attr, W1, b1, W2, b2):
    in_maps = _shard_inputs(z, edge_index, edge_attr, W1, b1, W2, b2)
    nc = build_nc()
    res = run_bass_kernel_spmd(nc, in_maps, core_ids=list(range(N_CORES))).results
    outT = np.concatenate([res[c]["outT"] for c in range(N_CORES)], axis=1)
    return np.ascontiguousarray(outT.T[:E_FULL])



# revision 2
# speedup vs baseline: 1.1522x; 1.1522x over previous
"""GCN joint-representation edge MLP on 8 TRN2 NeuronCores (Bass/Tile).

reference:
    node_rep = z[edge_index[0]] * z[edge_index[1]]          # [E, 64]
    joint    = concat([node_rep, edge_attr], -1)            # [E, 832]
    h        = relu(joint @ W1 + b1)                        # [E, 128]
    out      = softmax(h @ W2 + b2, -1)                     # [E, 5]

Sharding: pure data-parallel over edges, 8 cores x 25088 edges (E padded
200000 -> 200704).  Each core streams its edge slice and runs the full
MLP + softmax on device.

Layout choices made during host-side sharding:
  - endpoint z-rows are resolved to dense per-edge streams (z[src], z[dst]).
    Device-side row-gather primitives are unusable in this runtime
    (multi-offset indirect DMA returns wrong data on HW; the dma_gather
    GPSIMD ucode crashes the exec unit; per-128-row indirect DMA costs
    1.6us/call = 3x the whole kernel budget).  The dense streams carry
    byte-for-byte the same device traffic as an on-device gather would.
  - all per-edge streams are bf16 (tolerance 2e-2; measured ~6e-3) and
    feature-major: matmul operands DMA straight into [K, N] layout, the
    output is written class-major [5, E] — zero on-chip transposes.
  - attr is packed [128, nblk, 6, 512] so each block's DMA is one
    contiguous 6 KiB run per partition.

Device pipeline per 512-edge block (all edges in natural order):
  - DMA attr [128, 6, 512] bf16 (SP ring), zz [64, 1024] bf16 (ACT ring)
  - node_rep = zz[:, :512]*zz[:, 512:] (DVE, bf16 out)     [64, 512]
  - 7 accumulating bf16 matmuls -> hT (PSUM f32)           [128, 512]
  - ScalarE relu(+b1) -> hT bf16
  - matmul lhsT=W2 rhs=hT -> logitsT (PSUM f32)            [5, 512]
  - ScalarE exp(logitsT + b2) -> bf16                      [5, 512]
  - matmul lhsT=ones[5,1] -> class sums (PSUM f32)         [1, 512]
  - DVE reciprocal + partition-broadcast multiply -> probsT [5, 512] f32
  - DMA probsT -> outT[5, E] (ACT ring)
"""
import numpy as np

import concourse.bass as bass
import concourse.bacc as bacc
import concourse.tile as tile
from concourse import mybir
from concourse.bass_utils import run_bass_kernel_spmd

F32 = mybir.dt.float32
BF16 = mybir.dt.bfloat16

N_CORES = 8
E_FULL = 200000
E_PAD = 200704              # 8 * 25088
E_CORE = E_PAD // N_CORES   # 25088 = 49 * 512
BLK = 512
NBLK = E_CORE // BLK        # 49
ZD = 64
AD = 768
NSL = AD // 128             # 6 attr feature slices
HID = 128
NCLS = 5


def build_nc(nblk=NBLK, reps=1):
    """Per-core Bass program (same NEFF on all 8 cores).  `reps` wraps the
    block loop with a For_i for timing runs."""
    nc = bacc.Bacc("TRN2", target_bir_lowering=False, debug=False)

    ecore = nblk * BLK
    attrP = nc.declare_dram_parameter("attrP", [128, nblk * NSL * BLK], BF16,
                                      isOutput=False)
    zzP = nc.declare_dram_parameter("zzP", [ZD, 2 * ecore], BF16, isOutput=False)
    w1a = nc.declare_dram_parameter("w1a", [ZD, HID], BF16, isOutput=False)
    w1f = nc.declare_dram_parameter("w1f", [128, NSL, HID], BF16, isOutput=False)
    w2 = nc.declare_dram_parameter("w2", [HID, NCLS], BF16, isOutput=False)
    b1 = nc.declare_dram_parameter("b1", [HID, 1], F32, isOutput=False)
    b2c = nc.declare_dram_parameter("b2c", [NCLS, 1], F32, isOutput=False)
    outT = nc.declare_dram_parameter("outT", [NCLS, ecore], F32, isOutput=True)

    attrP_v = attrP[:, :].rearrange("p (b s e) -> b p s e", s=NSL, e=BLK)
    zzP_v = zzP[:, :].rearrange("p (b e) -> b p e", e=2 * BLK)
    outT_v = outT[:, :].rearrange("p (b e) -> b p e", e=BLK)

    with tile.TileContext(nc) as tc:
        with (
            tc.tile_pool(name="const", bufs=1) as constp,
            tc.tile_pool(name="attrp", bufs=3) as attrp,
            tc.tile_pool(name="zp", bufs=3) as zp,
            tc.tile_pool(name="nrp", bufs=2) as nrp,
            tc.tile_pool(name="htp", bufs=2) as htp,
            tc.tile_pool(name="exp_", bufs=3) as expp,
            tc.tile_pool(name="outp", bufs=3) as outp,
            tc.tile_pool(name="ps_ht", bufs=2, space="PSUM") as ps_ht,
            tc.tile_pool(name="ps_lg", bufs=2, space="PSUM") as ps_lg,
            tc.tile_pool(name="ps_sum", bufs=2, space="PSUM") as ps_sum,
        ):
            # ---- constants ----
            w1a_t = constp.tile([ZD, HID], BF16)
            nc.sync.dma_start(out=w1a_t[:], in_=w1a[:, :])
            w1f_t = constp.tile([128, NSL, HID], BF16)
            nc.sync.dma_start(out=w1f_t[:], in_=w1f[:, :, :])
            w2_t = constp.tile([HID, NCLS], BF16)
            nc.sync.dma_start(out=w2_t[:], in_=w2[:, :])
            b1_t = constp.tile([HID, 1], F32)
            nc.sync.dma_start(out=b1_t[:], in_=b1[:, :])
            b2_t = constp.tile([NCLS, 1], F32)
            nc.sync.dma_start(out=b2_t[:], in_=b2c[:, :])
            ones_t = constp.tile([NCLS, 1], BF16)
            nc.vector.memset(ones_t[:], 1.0)
            ones1_t = constp.tile([1, NCLS], F32)
            nc.vector.memset(ones1_t[:], 1.0)

            def body(b):
                attr_t = attrp.tile([128, NSL, BLK], BF16, tag="attr")
                nc.sync.dma_start(out=attr_t[:], in_=attrP_v[b])
                zz_t = zp.tile([ZD, 2 * BLK], BF16, tag="zz")
                nc.scalar.dma_start(out=zz_t[:], in_=zzP_v[b])

                nr_t = nrp.tile([ZD, BLK], BF16, tag="nr")
                nc.vector.tensor_mul(nr_t[:], zz_t[:, 0:BLK], zz_t[:, BLK:2 * BLK])

                # ---- layer 1: hT[128, 512] ----
                ht_ps = ps_ht.tile([HID, BLK], F32, tag="htps")
                nc.tensor.matmul(
                    out=ht_ps[:], lhsT=w1a_t[:], rhs=nr_t[:],
                    start=True, stop=False,
                )
                for s in range(NSL):
                    nc.tensor.matmul(
                        out=ht_ps[:], lhsT=w1f_t[:, s, :], rhs=attr_t[:, s, :],
                        start=False, stop=(s == NSL - 1),
                    )

                # ---- relu(+b1) -> hT bf16 ----
                ht_s = htp.tile([HID, BLK], BF16, tag="hts")
                nc.scalar.activation(
                    out=ht_s[:], in_=ht_ps[:],
                    func=mybir.ActivationFunctionType.Relu,
                    bias=b1_t[:],
                )

                # ---- layer 2: logitsT [5, 512] ----
                lg_ps = ps_lg.tile([NCLS, BLK], F32, tag="lgps")
                nc.tensor.matmul(
                    out=lg_ps[:], lhsT=w2_t[:], rhs=ht_s[:],
                    start=True, stop=True,
                )
                # exp(logits + b2) -> bf16
                ex_t = expp.tile([NCLS, BLK], BF16, tag="ex")
                nc.scalar.activation(
                    out=ex_t[:], in_=lg_ps[:],
                    func=mybir.ActivationFunctionType.Exp,
                    bias=b2_t[:],
                )
                # class sums via ones-matmul -> [1, 512]
                sum_ps = ps_sum.tile([1, BLK], F32, tag="sumps")
                nc.tensor.matmul(
                    out=sum_ps[:], lhsT=ones_t[:], rhs=ex_t[:],
                    start=True, stop=True,
                )
                rec = expp.tile([1, BLK], F32, tag="rec")
                nc.vector.reciprocal(out=rec[:], in_=sum_ps[:])
                # broadcast rec across the 5 class partitions via K=1 matmul
                rec5_ps = ps_sum.tile([NCLS, BLK], F32, tag="rec5")
                nc.tensor.matmul(
                    out=rec5_ps[:], lhsT=ones1_t[:], rhs=rec[:],
                    start=True, stop=True,
                )
                pr_t = outp.tile([NCLS, BLK], F32, tag="pr")
                nc.vector.tensor_mul(pr_t[:], ex_t[:], rec5_ps[:])
                nc.scalar.dma_start(out=outT_v[b], in_=pr_t[:])

            if reps == 1:
                for b in range(nblk):
                    body(b)
            else:
                with tc.For_i(0, reps, 1):
                    for b in range(nblk):
                        body(b)

    nc.compile()
    return nc


def _shard_inputs(z, edge_index, edge_attr, W1, b1, W2, b2):
    import ml_dtypes
    bf = ml_dtypes.bfloat16
    z = np.asarray(z, dtype=np.float32)
    ei = np.asarray(edge_index).astype(np.int64)
    attr = np.asarray(edge_attr, dtype=np.float32)
    W1 = np.asarray(W1, dtype=np.float32)
    b1 = np.asarray(b1, dtype=np.float32)
    W2 = np.asarray(W2, dtype=np.float32)
    b2 = np.asarray(b2, dtype=np.float32)

    src = np.zeros(E_PAD, dtype=np.int64)
    dst = np.zeros(E_PAD, dtype=np.int64)
    src[:E_FULL] = ei[0]
    dst[:E_FULL] = ei[1]

    zb = z.astype(bf)
    # dense per-edge endpoint streams, feature-major, per-block [zs512|zd512]
    nblk_tot = E_PAD // BLK
    zzP = np.empty((ZD, nblk_tot, 2, BLK), dtype=bf)
    zzP[:, :, 0, :] = zb[src].T.reshape(ZD, nblk_tot, BLK)
    zzP[:, :, 1, :] = zb[dst].T.reshape(ZD, nblk_tot, BLK)
    zzP = zzP.reshape(ZD, 2 * E_PAD)

    # attr packed [128, nblk, 6, 512]: [p, b, s, e] = attr[b*512+e, s*128+p]
    attrP = np.zeros((AD, E_PAD), dtype=bf)
    attrP[:, :E_FULL] = attr.T.astype(bf)
    attrP = np.ascontiguousarray(
        attrP.reshape(NSL, 128, nblk_tot, BLK).transpose(1, 2, 0, 3))

    w1a = W1[:ZD].astype(bf)                      # [64, 128] node_rep rows
    w1f = np.ascontiguousarray(
        W1[ZD:].reshape(NSL, 128, HID).transpose(1, 0, 2)).astype(bf)
    w2b = W2.astype(bf)
    b1c = b1.reshape(HID, 1)
    b2c = b2.reshape(NCLS, 1)

    nb_core = NBLK
    in_maps = []
    for c in range(N_CORES):
        s = slice(c * nb_core, (c + 1) * nb_core)
        s2 = slice(2 * c * E_CORE, 2 * (c + 1) * E_CORE)
        in_maps.append({
            "attrP": np.ascontiguousarray(
                attrP[:, s]).reshape(128, nb_core * NSL * BLK),
            "zzP": np.ascontiguousarray(zzP[:, s2]),
            "w1a": w1a,
            "w1f": w1f,
            "w2": w2b,
            "b1": b1c,
            "b2c": b2c,
        })
    return in_maps


def kernel(z, edge_index, edge_attr, W1, b1, W2, b2):
    in_maps = _shard_inputs(z, edge_index, edge_attr, W1, b1, W2, b2)
    nc = build_nc()
    res = run_bass_kernel_spmd(nc, in_maps, core_ids=list(range(N_CORES))).results
    outT = np.concatenate([res[c]["outT"] for c in range(N_CORES)], axis=1)
    return np.ascontiguousarray(outT.T[:E_FULL])


# revision 9
# speedup vs baseline: 1.7448x; 1.5143x over previous
"""GCN joint-representation edge MLP on 8 TRN2 NeuronCores (Bass/Tile).

reference:
    node_rep = z[edge_index[0]] * z[edge_index[1]]          # [E, 64]
    joint    = concat([node_rep, edge_attr], -1)            # [E, 832]
    h        = relu(joint @ W1 + b1)                        # [E, 128]
    out      = softmax(h @ W2 + b2, -1)                     # [E, 5]

Sharding: pure data-parallel over edges, 8 cores x 25088 edges (E padded
200000 -> 200704).  Each core streams its edge slice and runs the full
MLP + softmax on device.

Layout choices made during host-side sharding:
  - endpoint z-rows are resolved to dense per-edge streams (z[src], z[dst]).
    Device-side row-gather primitives are unusable in this runtime
    (multi-offset indirect DMA returns wrong data on HW; the dma_gather
    GPSIMD ucode crashes the exec unit; per-128-row indirect DMA costs
    1.6us/call = 3x the whole kernel budget).  The dense streams carry
    byte-for-byte the same device traffic as an on-device gather would.
  - all per-edge streams are bf16 (tolerance 2e-2; measured ~5e-3) and
    feature-major: matmul operands DMA straight into [K, N] layout, the
    output is written class-major [5, E] bf16 — zero on-chip transposes.
  - attr is packed [128, nblk, 6, 512] so each block's DMA is one
    contiguous 6 KiB run per partition.

Engine queues on TRN2 are strict FIFO, so the softmax tail (a serial
PE->ACT->PE->DVE->PE->DVE chain) must not sit between the layer-1 matmul
bursts of consecutive blocks: a tail op waiting at an engine's queue head
convoys everything behind it (measured 4.9us/block for the tail alone).
The block loop is therefore software-pipelined with a 3-deep skew; at
iteration b each engine only pops ops whose inputs finished >= 1 iteration
ago:
  stage A (block b):   DMA attr/zz, DVE nr=zs*zd, PE 7 matmuls -> ht_ps
  stage B (block b-1): ACT relu(+b1)->bf16, PE lg matmul, ACT exp(+b2)
  stage C (block b-2): PE ones-matmul class sums, DVE reciprocal (bf16)
  stage D (block b-3): PE K=1 broadcast matmul, DVE multiply, DMA out
"""
import numpy as np

import concourse.bass as bass
import concourse.bacc as bacc
import concourse.tile as tile
from concourse import mybir
from concourse.bass_utils import run_bass_kernel_spmd

F32 = mybir.dt.float32
BF16 = mybir.dt.bfloat16

N_CORES = 8
E_FULL = 200000
E_PAD = 200704              # 8 * 25088
E_CORE = E_PAD // N_CORES   # 25088 = 49 * 512
BLK = 512
NBLK = E_CORE // BLK        # 49
ZD = 64
AD = 768
NSL = AD // 128             # 6 attr feature slices
HID = 128
NCLS = 5


def build_nc(nblk=NBLK, reps=1, mode="full"):
    """Per-core Bass program (same NEFF on all 8 cores).  `reps` wraps the
    block loop with a For_i for timing runs.  mode: full | dma | compute
    (dma = streams only, compute = engines only; for HW bisection probes)."""
    nc = bacc.Bacc("TRN2", target_bir_lowering=False, debug=False)

    ecore = nblk * BLK
    attrP = nc.declare_dram_parameter("attrP", [128, nblk * NSL * BLK], BF16,
                                      isOutput=False)
    zzP = nc.declare_dram_parameter("zzP", [ZD, 2 * ecore], BF16, isOutput=False)
    w1a = nc.declare_dram_parameter("w1a", [ZD, HID], BF16, isOutput=False)
    w1f = nc.declare_dram_parameter("w1f", [128, NSL, HID], BF16, isOutput=False)
    w2 = nc.declare_dram_parameter("w2", [HID, NCLS], BF16, isOutput=False)
    b1 = nc.declare_dram_parameter("b1", [HID, 1], F32, isOutput=False)
    b2c = nc.declare_dram_parameter("b2c", [NCLS, 1], F32, isOutput=False)
    outT = nc.declare_dram_parameter("outT", [NCLS, ecore], BF16, isOutput=True)

    attrP_v = attrP[:, :].rearrange("p (b s e) -> b p s e", s=NSL, e=BLK)
    zzP_v = zzP[:, :].rearrange("p (b e) -> b p e", e=2 * BLK)
    outT_v = outT[:, :].rearrange("p (b e) -> b p e", e=BLK)

    with tile.TileContext(nc) as tc:
        with (
            tc.tile_pool(name="const", bufs=1) as constp,
            tc.tile_pool(name="attrp", bufs=3) as attrp,
            tc.tile_pool(name="zp", bufs=3) as zp,
            tc.tile_pool(name="nrp", bufs=2) as nrp,
            tc.tile_pool(name="htp", bufs=2) as htp,
            tc.tile_pool(name="exp_", bufs=3) as expp,
            tc.tile_pool(name="recp", bufs=2) as recp,
            tc.tile_pool(name="outp", bufs=3) as outp,
            tc.tile_pool(name="ps_ht", bufs=2, space="PSUM") as ps_ht,
            tc.tile_pool(name="ps_lg", bufs=2, space="PSUM") as ps_lg,
            tc.tile_pool(name="ps_sum", bufs=2, space="PSUM") as ps_sum,
            tc.tile_pool(name="ps_rec", bufs=2, space="PSUM") as ps_rec,
        ):
            # ---- constants ----
            w1a_t = constp.tile([ZD, HID], BF16)
            nc.sync.dma_start(out=w1a_t[:], in_=w1a[:, :])
            w1f_t = constp.tile([128, NSL, HID], BF16)
            nc.sync.dma_start(out=w1f_t[:], in_=w1f[:, :, :])
            w2_t = constp.tile([HID, NCLS], BF16)
            nc.sync.dma_start(out=w2_t[:], in_=w2[:, :])
            b1_t = constp.tile([HID, 1], F32)
            nc.sync.dma_start(out=b1_t[:], in_=b1[:, :])
            b2_t = constp.tile([NCLS, 1], F32)
            nc.sync.dma_start(out=b2_t[:], in_=b2c[:, :])
            ones_t = constp.tile([NCLS, 1], BF16)
            nc.vector.memset(ones_t[:], 1.0)
            ones1_t = constp.tile([1, NCLS], BF16)
            nc.vector.memset(ones1_t[:], 1.0)

            if mode in ("compute", "l1", "tail"):
                attr_c = constp.tile([128, NSL, BLK], BF16)
                nc.vector.memset(attr_c[:], 0.25)
                zz_c = constp.tile([ZD, 2 * BLK], BF16)
                nc.vector.memset(zz_c[:], 0.5)
                ht_c = constp.tile([HID, BLK], BF16)
                nc.vector.memset(ht_c[:], 0.5)
            if mode == "dma":
                pr_c = constp.tile([NCLS, BLK], BF16)
                nc.vector.memset(pr_c[:], 0.125)

            # rolling per-stage state, keyed by block index
            live = {}

            def stageA(b):
                if mode in ("compute", "l1", "tail"):
                    attr_t, zz_t = attr_c, zz_c
                else:
                    attr_t = attrp.tile([128, NSL, BLK], BF16, tag="attr")
                    nc.sync.dma_start(out=attr_t[:], in_=attrP_v[b])
                    zz_t = zp.tile([ZD, 2 * BLK], BF16, tag="zz")
                    nc.scalar.dma_start(out=zz_t[:], in_=zzP_v[b])
                if mode == "dma":
                    nc.scalar.dma_start(out=outT_v[b], in_=pr_c[:])
                    return
                if mode == "tail":
                    return
                nr_t = nrp.tile([ZD, BLK], BF16, tag="nr")
                nc.vector.tensor_mul(nr_t[:], zz_t[:, 0:BLK], zz_t[:, BLK:2 * BLK])
                ht_ps = ps_ht.tile([HID, BLK], F32, tag="htps")
                nc.tensor.matmul(out=ht_ps[:], lhsT=w1a_t[:], rhs=nr_t[:],
                                 start=True, stop=False)
                for s in range(NSL):
                    nc.tensor.matmul(out=ht_ps[:], lhsT=w1f_t[:, s, :],
                                     rhs=attr_t[:, s, :],
                                     start=False, stop=(s == NSL - 1))
                live[("ht_ps", b)] = ht_ps

            def stageB(b):
                if mode == "tail":
                    ht_s = ht_c
                else:
                    ht_ps = live.pop(("ht_ps", b))
                    ht_s = htp.tile([HID, BLK], BF16, tag="hts")
                    nc.scalar.activation(out=ht_s[:], in_=ht_ps[:],
                                         func=mybir.ActivationFunctionType.Relu,
                                         bias=b1_t[:])
                    if mode == "l1":
                        return
                lg_ps = ps_lg.tile([NCLS, BLK], F32, tag="lgps")
                nc.tensor.matmul(out=lg_ps[:], lhsT=w2_t[:], rhs=ht_s[:],
                                 start=True, stop=True)
                ex_t = expp.tile([NCLS, BLK], BF16, tag="ex")
                nc.scalar.activation(out=ex_t[:], in_=lg_ps[:],
                                     func=mybir.ActivationFunctionType.Exp,
                                     bias=b2_t[:])
                live[("ex", b)] = ex_t

            def stageC(b):
                ex_t = live[("ex", b)]
                sum_ps = ps_sum.tile([1, BLK], F32, tag="sumps")
                nc.tensor.matmul(out=sum_ps[:], lhsT=ones_t[:], rhs=ex_t[:],
                                 start=True, stop=True)
                rec = recp.tile([1, BLK], BF16, tag="rec")
                with nc.allow_low_precision(reason="1/sum in bf16: 0.4% on probs"):
                    nc.vector.reciprocal(out=rec[:], in_=sum_ps[:])
                live[("rec", b)] = rec

            def stageD(b):
                ex_t = live.pop(("ex", b))
                rec = live.pop(("rec", b))
                rec5_ps = ps_rec.tile([NCLS, BLK], F32, tag="rec5")
                nc.tensor.matmul(out=rec5_ps[:], lhsT=ones1_t[:], rhs=rec[:],
                                 start=True, stop=True)
                pr_t = outp.tile([NCLS, BLK], BF16, tag="pr")
                nc.vector.tensor_mul(pr_t[:], ex_t[:], rec5_ps[:])
                if mode != "compute":
                    nc.scalar.dma_start(out=outT_v[b], in_=pr_t[:])

            def body():
                for b in range(nblk + 3):
                    if b < nblk:
                        stageA(b)
                    if mode in ("dma",):
                        continue
                    if mode == "l1":
                        if 1 <= b:
                            if b - 1 < nblk:
                                stageB(b - 1)
                        continue
                    if 1 <= b <= nblk and b - 1 < nblk:
                        stageB(b - 1)
                    if 2 <= b and 0 <= b - 2 < nblk:
                        stageC(b - 2)
                    if 3 <= b and 0 <= b - 3 < nblk:
                        stageD(b - 3)

            if reps == 1:
                body()
            else:
                with tc.For_i(0, reps, 1):
                    body()

    nc.compile()
    return nc


def _shard_inputs(z, edge_index, edge_attr, W1, b1, W2, b2):
    import ml_dtypes
    bf = ml_dtypes.bfloat16
    z = np.asarray(z, dtype=np.float32)
    ei = np.asarray(edge_index).astype(np.int64)
    attr = np.asarray(edge_attr, dtype=np.float32)
    W1 = np.asarray(W1, dtype=np.float32)
    b1 = np.asarray(b1, dtype=np.float32)
    W2 = np.asarray(W2, dtype=np.float32)
    b2 = np.asarray(b2, dtype=np.float32)

    src = np.zeros(E_PAD, dtype=np.int64)
    dst = np.zeros(E_PAD, dtype=np.int64)
    src[:E_FULL] = ei[0]
    dst[:E_FULL] = ei[1]

    zb = z.astype(bf)
    # dense per-edge endpoint streams, feature-major, per-block [zs512|zd512]
    nblk_tot = E_PAD // BLK
    zzP = np.empty((ZD, nblk_tot, 2, BLK), dtype=bf)
    zzP[:, :, 0, :] = zb[src].T.reshape(ZD, nblk_tot, BLK)
    zzP[:, :, 1, :] = zb[dst].T.reshape(ZD, nblk_tot, BLK)
    zzP = zzP.reshape(ZD, 2 * E_PAD)

    # attr packed [128, nblk, 6, 512]: [p, b, s, e] = attr[b*512+e, s*128+p]
    attrP = np.zeros((AD, E_PAD), dtype=bf)
    attrP[:, :E_FULL] = attr.T.astype(bf)
    attrP = np.ascontiguousarray(
        attrP.reshape(NSL, 128, nblk_tot, BLK).transpose(1, 2, 0, 3))

    w1a = W1[:ZD].astype(bf)                      # [64, 128] node_rep rows
    w1f = np.ascontiguousarray(
        W1[ZD:].reshape(NSL, 128, HID).transpose(1, 0, 2)).astype(bf)
    w2b = W2.astype(bf)
    b1c = b1.reshape(HID, 1)
    b2c = b2.reshape(NCLS, 1)

    nb_core = NBLK
    in_maps = []
    for c in range(N_CORES):
        s = slice(c * nb_core, (c + 1) * nb_core)
        s2 = slice(2 * c * E_CORE, 2 * (c + 1) * E_CORE)
        in_maps.append({
            "attrP": np.ascontiguousarray(
                attrP[:, s]).reshape(128, nb_core * NSL * BLK),
            "zzP": np.ascontiguousarray(zzP[:, s2]),
            "w1a": w1a,
            "w1f": w1f,
            "w2": w2b,
            "b1": b1c,
            "b2c": b2c,
        })
    return in_maps


def kernel(z, edge_index, edge_attr, W1, b1, W2, b2):
    in_maps = _shard_inputs(z, edge_index, edge_attr, W1, b1, W2, b2)
    nc = build_nc()
    res = run_bass_kernel_spmd(nc, in_maps, core_ids=list(range(N_CORES))).results
    outT = np.concatenate([res[c]["outT"] for c in range(N_CORES)], axis=1)
    return np.ascontiguousarray(outT.T[:E_FULL].astype(np.float32))


# revision 10
# speedup vs baseline: 2.9879x; 1.7124x over previous
"""GCN joint-representation edge MLP on 8 TRN2 NeuronCores (Bass/Tile).

reference:
    node_rep = z[edge_index[0]] * z[edge_index[1]]          # [E, 64]
    joint    = concat([node_rep, edge_attr], -1)            # [E, 832]
    h        = relu(joint @ W1 + b1)                        # [E, 128]
    out      = softmax(h @ W2 + b2, -1)                     # [E, 5]

Sharding: pure data-parallel over edges, 8 cores x 25088 edges (E padded
200000 -> 200704).  Each core streams its edge slice and runs the full
MLP + softmax on device.

Layout choices made during host-side sharding:
  - endpoint z-rows are resolved to dense per-edge streams (z[src], z[dst]).
    Device-side row-gather primitives are unusable in this runtime
    (multi-offset indirect DMA returns wrong data on HW; the dma_gather
    GPSIMD ucode crashes the exec unit; per-128-row indirect DMA costs
    1.6us/call = 3x the whole kernel budget).  The dense streams carry
    byte-for-byte the same device traffic as an on-device gather would.
  - all per-edge streams are bf16 (tolerance 2e-2; measured ~7e-3) and
    feature-major for layer 1: matmul operands DMA straight into [K, N]
    layout.  attr is packed [128, nblk, 6, 512] so each block's DMA is one
    contiguous 6 KiB run per partition.

Engine queues on TRN2 are strict FIFO and every cross-engine semaphore
handoff costs ~0.5-1us of queue-head latency, so the classic class-major
softmax tail (PE->ACT->PE->DVE->PE->DVE of tiny ops per block) runs at
~4.7us/block no matter how it's overlapped.  Instead:
  - layer 2 is computed EDGE-major: 4 matmuls per block with lhsT =
    hT[:, 128-edge chunk] (the stationary operand) and rhs = W2[128, 5],
    writing logits [128 edges, 5 classes] into one shared PSUM bank
    (class stride padded to 8 for cacheline alignment).
  - softmax then runs along the FREE dim, batched over 7-block groups:
    one ACT exp [128, 196B], one DVE add-reduce over classes, one DVE
    reciprocal, one DVE broadcast multiply per ~3600 edges.
  - probs accumulate in one SBUF tile [128, nblk*20] bf16 (edge-major
    [E, 5] modulo the block/chunk interleave) and leave in a single DMA.

Software-pipeline skew (engines only pop ops whose inputs finished >=1
iteration ago): iter b runs DMA+nr+layer1 for block b, relu + 4 layer-2
matmuls for b-1, and the batched softmax for the group that ended at b-2.
"""
import numpy as np

import concourse.bass as bass
import concourse.bacc as bacc
import concourse.tile as tile
from concourse import mybir
from concourse.bass_utils import run_bass_kernel_spmd

F32 = mybir.dt.float32
BF16 = mybir.dt.bfloat16

N_CORES = 8
E_FULL = 200000
E_PAD = 200704              # 8 * 25088
E_CORE = E_PAD // N_CORES   # 25088 = 49 * 512
BLK = 512
NBLK = E_CORE // BLK        # 49
ZD = 64
AD = 768
NSL = AD // 128             # 6 attr feature slices
HID = 128
NCLS = 5
NCH = BLK // 128            # 4 edge chunks per block for layer 2
GRP = 7                     # blocks per softmax group (49 = 7*7)


def build_nc(nblk=NBLK, reps=1, mode="full", has_b2=False):
    """Per-core Bass program (same NEFF on all 8 cores).  `reps` wraps the
    block loop with a For_i for timing runs.  mode: full | dma | l1
    (bisection probes)."""
    nc = bacc.Bacc("TRN2", target_bir_lowering=False, debug=False)

    ecore = nblk * BLK
    ngrp = (nblk + GRP - 1) // GRP
    attrP = nc.declare_dram_parameter("attrP", [128, nblk * NSL * BLK], BF16,
                                      isOutput=False)
    zzP = nc.declare_dram_parameter("zzP", [ZD, 2 * ecore], BF16, isOutput=False)
    w1a = nc.declare_dram_parameter("w1a", [ZD, HID], BF16, isOutput=False)
    w1f = nc.declare_dram_parameter("w1f", [128, NSL, HID], BF16, isOutput=False)
    w2 = nc.declare_dram_parameter("w2", [HID, NCLS], BF16, isOutput=False)
    b1 = nc.declare_dram_parameter("b1", [HID, 1], F32, isOutput=False)
    if has_b2:
        eb2 = nc.declare_dram_parameter("eb2", [128, NCLS], F32, isOutput=False)
    # edge-major probs: [p, b*NCH*NCLS] with edge = b*512 + c*128 + p
    outE = nc.declare_dram_parameter("outE", [128, nblk * NCH * NCLS], BF16,
                                     isOutput=True)

    attrP_v = attrP[:, :].rearrange("p (b s e) -> b p s e", s=NSL, e=BLK)
    zzP_v = zzP[:, :].rearrange("p (b e) -> b p e", e=2 * BLK)

    with tile.TileContext(nc) as tc:
        with (
            tc.tile_pool(name="const", bufs=1) as constp,
            tc.tile_pool(name="attrp", bufs=3) as attrp,
            tc.tile_pool(name="zp", bufs=3) as zp,
            tc.tile_pool(name="nrp", bufs=2) as nrp,
            tc.tile_pool(name="htp", bufs=2) as htp,
            tc.tile_pool(name="exg", bufs=2) as exgp,
            tc.tile_pool(name="recg", bufs=2) as recgp,
            tc.tile_pool(name="ps_ht", bufs=3, space="PSUM") as ps_ht,
            tc.tile_pool(name="ps_lg", bufs=2, space="PSUM") as ps_lg,
        ):
            # ---- constants ----
            w1a_t = constp.tile([ZD, HID], BF16)
            nc.sync.dma_start(out=w1a_t[:], in_=w1a[:, :])
            w1f_t = constp.tile([128, NSL, HID], BF16)
            nc.sync.dma_start(out=w1f_t[:], in_=w1f[:, :, :])
            w2_t = constp.tile([HID, NCLS], BF16)
            nc.sync.dma_start(out=w2_t[:], in_=w2[:, :])
            b1_t = constp.tile([HID, 1], F32)
            nc.sync.dma_start(out=b1_t[:], in_=b1[:, :])
            if has_b2:
                eb2_t = constp.tile([128, NCLS], F32)
                nc.sync.dma_start(out=eb2_t[:], in_=eb2[:, :])
            out_t = constp.tile([128, nblk * NCH * NCLS], BF16)

            if mode == "dma":
                nc.vector.memset(out_t[:], 0.125)

            live = {}

            def stageA(b):
                attr_t = attrp.tile([128, NSL, BLK], BF16, tag="attr")
                nc.sync.dma_start(out=attr_t[:], in_=attrP_v[b])
                zz_t = zp.tile([ZD, 2 * BLK], BF16, tag="zz")
                nc.scalar.dma_start(out=zz_t[:], in_=zzP_v[b])
                if mode == "dma":
                    return
                nr_t = nrp.tile([ZD, BLK], BF16, tag="nr")
                nc.vector.tensor_mul(nr_t[:], zz_t[:, 0:BLK], zz_t[:, BLK:2 * BLK])
                ht_ps = ps_ht.tile([HID, BLK], F32, tag="htps")
                nc.tensor.matmul(out=ht_ps[:], lhsT=w1a_t[:], rhs=nr_t[:],
                                 start=True, stop=False)
                for s in range(NSL):
                    nc.tensor.matmul(out=ht_ps[:], lhsT=w1f_t[:, s, :],
                                     rhs=attr_t[:, s, :],
                                     start=False, stop=(s == NSL - 1))
                live[("ht_ps", b)] = ht_ps

            def stageB(b):
                ht_ps = live.pop(("ht_ps", b))
                ht_s = htp.tile([HID, BLK], BF16, tag="hts")
                nc.scalar.activation(out=ht_s[:], in_=ht_ps[:],
                                     func=mybir.ActivationFunctionType.Relu,
                                     bias=b1_t[:])
                if mode == "l1":
                    return
                j = b % GRP
                if j == 0:
                    lg_ps = ps_lg.tile([128, GRP * NCH, 8], F32, tag="lgps")
                    live[("lg_ps", b // GRP)] = lg_ps
                else:
                    lg_ps = live[("lg_ps", b // GRP)]
                for c in range(NCH):
                    nc.tensor.matmul(
                        out=lg_ps[:, j * NCH + c, 0:NCLS],
                        lhsT=ht_s[:, c * 128:(c + 1) * 128],
                        rhs=w2_t[:, :],
                        start=True, stop=True,
                    )

            def stageC(g, glen):
                lg_ps = live.pop(("lg_ps", g))
                na = glen * NCH
                ex_g = exgp.tile([128, GRP * NCH, NCLS], BF16, tag="exg")
                nc.scalar.activation(out=ex_g[:, 0:na, :], in_=lg_ps[:, 0:na, 0:NCLS],
                                     func=mybir.ActivationFunctionType.Exp)
                if has_b2:
                    eb2_b = eb2_t[:, :].rearrange("p (a c) -> p a c", a=1)
                    with nc.allow_low_precision(reason="softmax num in bf16"):
                        nc.vector.tensor_mul(ex_g[:, 0:na, :], ex_g[:, 0:na, :],
                                             eb2_b.broadcast_to([128, na, NCLS]))
                sum_g = recgp.tile([128, GRP * NCH], F32, tag="sumg")
                nc.vector.tensor_reduce(out=sum_g[:, 0:na], in_=ex_g[:, 0:na, :],
                                        axis=mybir.AxisListType.X,
                                        op=mybir.AluOpType.add)
                rec_g = recgp.tile([128, GRP * NCH], F32, tag="recg")
                nc.vector.reciprocal(out=rec_g[:, 0:na], in_=sum_g[:, 0:na])
                rec_b = rec_g[:, 0:na].rearrange("p (a c) -> p a c", c=1)
                out_v = out_t[:, g * GRP * NCH * NCLS:
                              (g * GRP + glen) * NCH * NCLS].rearrange(
                    "p (a c) -> p a c", c=NCLS)
                with nc.allow_low_precision(reason="probs in bf16"):
                    nc.vector.tensor_mul(out_v, ex_g[:, 0:na, :],
                                         rec_b.broadcast_to([128, na, NCLS]))

            def body():
                for b in range(nblk + 2):
                    if b < nblk:
                        stageA(b)
                    if mode == "dma":
                        continue
                    if 0 <= b - 1 < nblk:
                        stageB(b - 1)
                    if mode == "l1":
                        continue
                    gb = b - 2  # last block of a completed group?
                    if 0 <= gb < nblk and (gb % GRP == GRP - 1 or gb == nblk - 1):
                        stageC(gb // GRP, gb % GRP + 1)
                nc.sync.dma_start(out=outE[:, :], in_=out_t[:])

            if reps == 1:
                body()
            else:
                with tc.For_i(0, reps, 1):
                    body()

    nc.compile()
    return nc


def _shard_inputs(z, edge_index, edge_attr, W1, b1, W2, b2):
    import ml_dtypes
    bf = ml_dtypes.bfloat16
    z = np.asarray(z, dtype=np.float32)
    ei = np.asarray(edge_index).astype(np.int64)
    attr = np.asarray(edge_attr, dtype=np.float32)
    W1 = np.asarray(W1, dtype=np.float32)
    b1 = np.asarray(b1, dtype=np.float32)
    W2 = np.asarray(W2, dtype=np.float32)
    b2 = np.asarray(b2, dtype=np.float32)

    src = np.zeros(E_PAD, dtype=np.int64)
    dst = np.zeros(E_PAD, dtype=np.int64)
    src[:E_FULL] = ei[0]
    dst[:E_FULL] = ei[1]

    zb = z.astype(bf)
    # dense per-edge endpoint streams, feature-major, per-block [zs512|zd512]
    nblk_tot = E_PAD // BLK
    zzP = np.empty((ZD, nblk_tot, 2, BLK), dtype=bf)
    zzP[:, :, 0, :] = zb[src].T.reshape(ZD, nblk_tot, BLK)
    zzP[:, :, 1, :] = zb[dst].T.reshape(ZD, nblk_tot, BLK)
    zzP = zzP.reshape(ZD, 2 * E_PAD)

    # attr packed [128, nblk, 6, 512]: [p, b, s, e] = attr[b*512+e, s*128+p]
    attrP = np.zeros((AD, E_PAD), dtype=bf)
    attrP[:, :E_FULL] = attr.T.astype(bf)
    attrP = np.ascontiguousarray(
        attrP.reshape(NSL, 128, nblk_tot, BLK).transpose(1, 2, 0, 3))

    w1a = W1[:ZD].astype(bf)                      # [64, 128] node_rep rows
    w1f = np.ascontiguousarray(
        W1[ZD:].reshape(NSL, 128, HID).transpose(1, 0, 2)).astype(bf)
    w2b = W2.astype(bf)
    b1c = b1.reshape(HID, 1)
    has_b2 = bool(np.any(b2 != 0.0))
    eb2 = np.broadcast_to(np.exp(b2).astype(np.float32), (128, NCLS)).copy()

    nb_core = NBLK
    in_maps = []
    for c in range(N_CORES):
        s = slice(c * nb_core, (c + 1) * nb_core)
        s2 = slice(2 * c * E_CORE, 2 * (c + 1) * E_CORE)
        m = {
            "attrP": np.ascontiguousarray(
                attrP[:, s]).reshape(128, nb_core * NSL * BLK),
            "zzP": np.ascontiguousarray(zzP[:, s2]),
            "w1a": w1a,
            "w1f": w1f,
            "w2": w2b,
            "b1": b1c,
        }
        if has_b2:
            m["eb2"] = eb2
        in_maps.append(m)
    return in_maps, has_b2


def _unshard_out(res, nblk=NBLK):
    # outE[c][p, b*20 + ch*5 + k] = prob(edge = c*E_CORE + b*512 + ch*128 + p,
    #                                    class k)
    outs = []
    for c in range(N_CORES):
        o = np.asarray(res[c]["outE"], dtype=np.float32)
        o = o.reshape(128, nblk, NCH, NCLS).transpose(1, 2, 0, 3)
        outs.append(o.reshape(nblk * BLK, NCLS))
    return np.concatenate(outs, axis=0)[:E_FULL]


def kernel(z, edge_index, edge_attr, W1, b1, W2, b2):
    in_maps, has_b2 = _shard_inputs(z, edge_index, edge_attr, W1, b1, W2, b2)
    nc = build_nc(has_b2=has_b2)
    res = run_bass_kernel_spmd(nc, in_maps, core_ids=list(range(N_CORES))).results
    return np.ascontiguousarray(_unshard_out(res))
